# revision 1
# baseline (speedup 1.0000x reference)
"""Per-pixel dynamic 5x5 conv (KernelConv2d) + leaky-relu, data-parallel on 8 TRN2 cores.

Sharding: core i <- (n = i//2, h-half = i%2); each core computes out[n, :, h0:h0+128, :].
Per-core compute: out[c,h,w] = sum_{dy,dx} x[c, h+dy-2, w+dx-2] * k[c, dy*5+dx, h, w]
(replicate padding), then leaky_relu(0.2).

On-chip layout: partitions = 128 output rows. Each partition stores its own
5-row replicate-padded x window (per channel), duplicated at two byte
alignments (even/odd dx) so every tensor_tensor read is 4B-aligned and the
DVE 2x_1P fp16 packing mode applies. Kernels stream per-dy (5 taps at a
time) from DRAM in fully contiguous 2.6 MB DMAs. 25 fp16 multiplies +
24 fp16 adds per core on VectorE; leaky-relu fused into one
scalar_tensor_tensor (max(0.2*x, x)); output DMA'd contiguously.
"""

import os
from contextlib import ExitStack

import numpy as np

import concourse.bass as bass
import concourse.mybir as mybir
import concourse.tile as tile
from concourse.bass_utils import run_bass_kernel_spmd

N, C, H, W = 4, 8, 256, 256
K = 5
PAD = 2
NCORES = 8
HSH = H // 2            # 128 output rows per core
XW = 264                # stored row width (padded 260 -> 264 for alignment)
CD = mybir.dt.float16   # compute/storage dtype on chip
NEG = 0.2

_NC_CACHE = {}


KCW = K * C * W          # 10240 fp16 kernel elems per partition per dy
XCW = 2 * C * XW         # 4224 fp16 x-window elems per partition per dy
ROW = KCW + XCW          # 14464


def _build_nc():
    nc = bass.Bass("TRN2", target_bir_lowering=False, debug=False,
                   num_devices=NCORES)
    # xk[p, dy, ROW]: per-partition, per-dy packed row = 5 kernel taps
    # (dx,c,w) followed by the x window row (par,c,264cols). One DMA per dy.
    xk = nc.dram_tensor("xk", [HSH, K, ROW], CD, kind="ExternalInput").ap()
    out = nc.dram_tensor("out", [HSH, C, W], CD, kind="ExternalOutput").ap()

    # Raw bass (no TileContext): this walrus build allows only ONE sync-wait
    # per instruction, so all waits are emitted as standalone wait ops and
    # each DMA gets its own semaphore (a shared sem's 16 per-engine incs
    # interleave across in-flight DMAs and would fire early).
    with ExitStack() as ctx:
        xkt = [ctx.enter_context(nc.sbuf_tensor(f"xkt{i}", [HSH, ROW], CD)) for i in range(K)]
        gt = [ctx.enter_context(nc.sbuf_tensor(f"gt{i}", [HSH, C, W], CD)) for i in range(K)]
        pt = ctx.enter_context(nc.sbuf_tensor("pt", [HSH, C, W], CD))
        ot = ctx.enter_context(nc.sbuf_tensor("ot", [HSH, C, W], CD))
        s_k = [ctx.enter_context(nc.semaphore(f"sk{i}")) for i in range(K)]
        s_v = ctx.enter_context(nc.semaphore("sv"))
        s_o = ctx.enter_context(nc.semaphore("so"))
        block = ctx.enter_context(nc.Block())

        @block.sync
        def _(sync):
            for dy in range(K):
                sync.dma_start(xkt[dy][:], xk[:, dy]).then_inc(s_k[dy], 16)
            sync.wait_ge(s_v, 1)
            sync.dma_start(out[:], ot[:]).then_inc(s_o, 16)
            sync.wait_ge(s_o, 16)

        @block.vector
        def _(vector):
            for dy in range(K):
                vector.wait_ge(s_k[dy], 16)
                kv = xkt[dy][:, :KCW].rearrange("p (t c w) -> p t c w", t=K, c=C)
                xv = xkt[dy][:, KCW:].rearrange("p (q c w) -> p q c w", q=2, c=C)
                g = gt[dy]
                for dx in range(K):
                    par = dx & 1
                    off = dx - par
                    xs = xv[:, par, :, off:off + W]
                    ks = kv[:, dx]
                    if dx == 0:
                        vector.tensor_tensor(g[:], xs, ks, op=mybir.AluOpType.mult)
                    else:
                        vector.tensor_tensor(pt[:], xs, ks, op=mybir.AluOpType.mult)
                        vector.tensor_tensor(g[:], g[:], pt[:], op=mybir.AluOpType.add)
            g0, g1, g2, g3, g4 = (g[:] for g in gt)
            vector.tensor_tensor(g0, g0, g1, op=mybir.AluOpType.add)
            vector.tensor_tensor(g2, g2, g3, op=mybir.AluOpType.add)
            vector.tensor_tensor(g0, g0, g2, op=mybir.AluOpType.add)
            # leaky_relu(x, 0.2) == max(0.2*x, x); last add then fused lrelu
            vector.tensor_tensor(g0, g0, g4, op=mybir.AluOpType.add)
            vector.scalar_tensor_tensor(ot[:], g0, NEG, g0,
                                        op0=mybir.AluOpType.mult,
                                        op1=mybir.AluOpType.max).then_inc(s_v, 1)
    return nc


def get_nc():
    if "nc" not in _NC_CACHE:
        _NC_CACHE["nc"] = _build_nc()
    return _NC_CACHE["nc"]


def _prep_shards(x: np.ndarray, kernel: np.ndarray):
    """Host-side: pad, cast to fp16, build per-core DMA-friendly layouts."""
    f16 = np.float16
    # rows: replicate 2 each side; cols: 2 left, 9 right (264-wide even view +
    # one extra col so the odd-shifted view is in range; tail cols never read)
    xp = np.pad(x, ((0, 0), (0, 0), (PAD, PAD), (PAD, XW + 1 - W - PAD)),
                mode='edge').astype(f16)  # (N, C, 260, 265)
    kr = kernel.reshape(N, C, K * K, H, W)

    in_maps = []
    for core in range(NCORES):
        n, hb = divmod(core, 2)
        h0 = hb * HSH
        # sliding 5-row windows: win[c, p, r, w] = xp[n, c, h0+p+r, w]
        win = np.lib.stride_tricks.sliding_window_view(
            xp[n, :, h0:h0 + HSH + K - 1, :], K, axis=1)  # (C, 128, 265, 5)
        win = win.transpose(1, 3, 0, 2)  # (128, 5, C, 265)
        xd = np.stack([win[..., 0:XW], win[..., 1:XW + 1]], axis=2)
        xd = xd.reshape(HSH, K, XCW)
        kt = kr[n, :, :, h0:h0 + HSH, :].transpose(2, 1, 0, 3).astype(f16)
        kt = kt.reshape(HSH, K, KCW)
        xkb = np.ascontiguousarray(
            np.concatenate([kt, xd], axis=2))  # (128, 5, ROW)
        in_maps.append({"xk": xkb})
    return in_maps


def kernel(x: np.ndarray, kernel: np.ndarray) -> np.ndarray:
    nc = get_nc()
    in_maps = _prep_shards(np.asarray(x), np.asarray(kernel))
    trace = bool(int(os.environ.get("KC_TRACE", "0")))
    res = run_bass_kernel_spmd(nc, in_maps, core_ids=list(range(NCORES)),
                               trace=trace)
    _NC_CACHE["last_results"] = res
    out = np.empty((N, C, H, W), np.float32)
    for core in range(NCORES):
        n, hb = divmod(core, 2)
        h0 = hb * HSH
        o = res.results[core]["out"]  # (128, C, W) fp16
        out[n, :, h0:h0 + HSH, :] = o.transpose(1, 0, 2).astype(np.float32)
    return out



# revision 2
# speedup vs baseline: 1.0178x; 1.0178x over previous
"""Per-pixel dynamic 5x5 conv (KernelConv2d) + leaky-relu, data-parallel on 8 TRN2 cores.

v4 architecture:
  Sharding: core i <- (n = i//2, h-half = i%2); each core computes out[n,:,h0:h0+128,:].
  On-chip layout: partitions = (c, w-block16); free = (w-in-block, h). The x tile
  stores a 4-col/4-row halo per partition ([128, 20, 132]) so every tap (dy,dx) is a
  free-dim shifted view -- x is DMA'd once (0.68 MB), no window copies.
  DVE does ONLY the 25 products (f16 tensor_tensor); the 24 adds run on the idle PE
  (identity-stationary matmuls accumulating in PSUM fp32; PE is pre-warmed with
  dummy matmuls to ramp its clock); the lrelu tail is ACT psum->f16 copy + one DVE
  scalar_tensor_tensor max(0.2g, g), split in halves to overlap the output DMA.
  The kernel stream is rate-matched to the DMA queue bus (22.5 B/ns x 16 queues):
  20 taps ship as f16 (DMA'd in pairs to keep the DMA/semaphore count and program
  preamble small) and 5 interspersed taps ship as int8 (half the bytes), upcast to
  f16 by the idle ACT engine with the dequant scale folded into the copy, so byte
  arrival keeps pace with DVE consumption. int8 rel-err ~5e-3 (gate 2e-2).
"""

import os
from contextlib import ExitStack

import numpy as np

import concourse.bass as bass
import concourse.mybir as mybir
from concourse.bass_utils import run_bass_kernel_spmd

N, C, H, W = 4, 8, 256, 256
K = 5
PAD = 2
NCORES = 8
HSH = H // 2            # 128 output rows per core
WB, WBW = 16, 16        # 16 w-blocks of 16 columns
XI, XJ = WBW + 2 * PAD, HSH + 2 * PAD   # 20 x 132 halo'd tile per partition
NT = K * K              # 25 taps
FREE = WBW * HSH        # 2048 product elems per partition
HALF = FREE // 2
BANK = 512              # PSUM fp32 bank width
RING = 10               # product ring buffers
WARM = 12               # PE warm-up matmuls

F16 = mybir.dt.float16
F32 = mybir.dt.float32
I8 = mybir.dt.int8

Q_TAPS = (4, 9, 14, 19, 22)         # int8 taps, upcast by ACT
F_TAPS = tuple(t for t in range(NT) if t not in Q_TAPS)
NF, NQ = len(F_TAPS), len(Q_TAPS)
F_GROUPS = [F_TAPS[i:i + 2] for i in range(0, NF, 2)]   # 10 pair-DMAs
NEG = 0.2

_NC_CACHE = {}


def _build_nc():
    nc = bass.Bass("TRN2", target_bir_lowering=False, debug=False,
                   num_devices=NCORES)
    xt_d = nc.dram_tensor("xt", [128, XI * XJ], F16, kind="ExternalInput").ap()
    id_d = nc.dram_tensor("ident", [128, 128], F16, kind="ExternalInput").ap()
    kf_d = nc.dram_tensor("kf", [128, NF, FREE], F16, kind="ExternalInput").ap()
    kq_d = nc.dram_tensor("kq", [128, NQ, FREE], I8, kind="ExternalInput").ap()
    out_d = nc.dram_tensor("out", [128, FREE], F16, kind="ExternalOutput").ap()

    taps = [(dy, dx) for dy in range(K) for dx in range(K)]
    fidx = {t: i for i, t in enumerate(F_TAPS)}
    qidx = {t: i for i, t in enumerate(Q_TAPS)}

    with ExitStack() as ctx:
        xt = ctx.enter_context(nc.sbuf_tensor("xts", [128, XI, XJ], F16))
        ident = ctx.enter_context(nc.sbuf_tensor("ids", [128, 128], F16))
        kbf = ctx.enter_context(nc.sbuf_tensor("kbf", [128, NF, FREE], F16))
        kb8 = ctx.enter_context(nc.sbuf_tensor("kb8", [128, NQ, FREE], I8))
        kbq = ctx.enter_context(nc.sbuf_tensor("kbq", [128, NQ, FREE], F16))
        pr = [ctx.enter_context(nc.sbuf_tensor(f"pr{r}", [128, FREE], F16))
              for r in range(RING)]
        gt = ctx.enter_context(nc.sbuf_tensor("gt", [128, FREE], F16))
        ot = ctx.enter_context(nc.sbuf_tensor("ot", [128, FREE], F16))
        ps = ctx.enter_context(nc.psum_tensor("ps", [128, FREE], F32))
        s_x = ctx.enter_context(nc.semaphore("s_x"))
        s_id = ctx.enter_context(nc.semaphore("s_id"))
        s_g = [ctx.enter_context(nc.semaphore(f"sg{i}"))
               for i in range(len(F_GROUPS))]
        s_q8 = [ctx.enter_context(nc.semaphore(f"sq{i}")) for i in range(NQ)]
        s_ku = ctx.enter_context(nc.semaphore("s_ku"))
        s_pv = ctx.enter_context(nc.semaphore("s_pv"))
        s_free = ctx.enter_context(nc.semaphore("s_free"))
        s_mm0 = ctx.enter_context(nc.semaphore("s_mm0"))
        s_mm1 = ctx.enter_context(nc.semaphore("s_mm1"))
        s_cp = ctx.enter_context(nc.semaphore("s_cp"))
        s_lr = ctx.enter_context(nc.semaphore("s_lr"))
        s_o = ctx.enter_context(nc.semaphore("s_o"))
        block = ctx.enter_context(nc.Block())

        @block.sync
        def _(sync):
            sync.dma_start(xt[:], xt_d.rearrange("p (i j) -> p i j", i=XI)) \
                .then_inc(s_x, 16)
            sync.dma_start(ident[:], id_d).then_inc(s_id, 16)
            # interleave: one int8 DMA after every second f16 pair
            qi = 0
            for g, pair in enumerate(F_GROUPS):
                f0 = fidx[pair[0]]
                sync.dma_start(kbf[:, f0:f0 + len(pair)],
                               kf_d[:, f0:f0 + len(pair)]) \
                    .then_inc(s_g[g], 16)
                if g % 2 == 1 and qi < NQ:
                    sync.dma_start(kb8[:, qi:qi + 1], kq_d[:, qi:qi + 1]) \
                        .then_inc(s_q8[qi], 16)
                    qi += 1
            sync.wait_ge(s_lr, 1)
            sync.dma_start(out_d[:, :HALF], ot[:, :HALF]).then_inc(s_o, 16)
            sync.wait_ge(s_lr, 2)
            sync.dma_start(out_d[:, HALF:], ot[:, HALF:]).then_inc(s_o, 16)
            sync.wait_ge(s_o, 32)

        @block.scalar
        def _(scalar):
            # preload the Copy act table off the critical path
            scalar.wait_ge(s_id, 16)
            scalar.activation(gt[:, :1], ident[:, :1],
                              mybir.ActivationFunctionType.Copy, scale=1.0)
            sq = float(_NC_CACHE.get("kscale", 1.0))
            for qi in range(NQ):
                scalar.wait_ge(s_q8[qi], 16)
                scalar.activation(kbq[:, qi], kb8[:, qi],
                                  mybir.ActivationFunctionType.Copy,
                                  scale=sq).then_inc(s_ku, 1)
            # tail: psum -> f16, in halves
            scalar.wait_ge(s_mm0, 1)
            scalar.activation(gt[:, :HALF], ps[:, :HALF],
                              mybir.ActivationFunctionType.Copy,
                              scale=1.0).then_inc(s_cp, 1)
            scalar.wait_ge(s_mm1, 1)
            scalar.activation(gt[:, HALF:], ps[:, HALF:],
                              mybir.ActivationFunctionType.Copy,
                              scale=1.0).then_inc(s_cp, 1)

        @block.vector
        def _(vector):
            vector.wait_ge(s_x, 16)
            for t, (dy, dx) in enumerate(taps):
                if t in qidx:
                    vector.wait_ge(s_ku, qidx[t] + 1)
                    kv = kbq[:, qidx[t]]
                else:
                    vector.wait_ge(s_g[fidx[t] // 2], 16)
                    kv = kbf[:, fidx[t]]
                if t >= RING:
                    vector.wait_ge(s_free, t - RING + 1)
                xs = xt[:, dx:dx + WBW, dy:dy + HSH]
                prv = pr[t % RING].ap().rearrange("p (i j) -> p i j", i=WBW)
                vector.tensor_tensor(prv, xs,
                                     kv.rearrange("p (i j) -> p i j", i=WBW),
                                     op=mybir.AluOpType.mult).then_inc(s_pv, 1)
            # lrelu tail halves: ot = max(NEG*gt, gt)
            for h in range(2):
                vector.wait_ge(s_cp, h + 1)
                sl = slice(h * HALF, (h + 1) * HALF)
                vector.scalar_tensor_tensor(ot[:, sl], gt[:, sl], NEG, gt[:, sl],
                                            op0=mybir.AluOpType.mult,
                                            op1=mybir.AluOpType.max) \
                    .then_inc(s_lr, 1)

        @block.tensor
        def _(tensor):
            tensor.wait_ge(s_id, 16)
            tensor.wait_ge(s_x, 16)
            for w in range(WARM):
                tensor.matmul(ps[:, 0:BANK], ident[:],
                              xt.ap().rearrange("p i j -> p (i j)")[:, 0:BANK],
                              start=True, stop=True)
            for t in range(NT):
                tensor.wait_ge(s_pv, t + 1)
                prf = pr[t % RING]
                for j in range(4):
                    mm = tensor.matmul(ps[:, j * BANK:(j + 1) * BANK], ident[:],
                                       prf[:, j * BANK:(j + 1) * BANK],
                                       start=(t == 0), stop=(t == NT - 1))
                    if t == NT - 1 and j == 1:
                        mm.then_inc(s_mm0, 1)
                if t == NT - 1:
                    mm.then_inc(s_mm1, 1)
                else:
                    mm.then_inc(s_free, 1)

    return nc


def get_nc():
    if "nc" not in _NC_CACHE:
        _NC_CACHE["nc"] = _build_nc()
    return _NC_CACHE["nc"]


def _prep_shards(x: np.ndarray, kernel: np.ndarray):
    f16 = np.float16
    xp = np.pad(x, ((0, 0), (0, 0), (PAD, PAD), (PAD, PAD)),
                mode='edge').astype(f16)          # (N, C, 260, 260)
    kr = kernel.reshape(N, C, NT, H, W)

    s = float(np.abs(kernel).max()) / 127.0
    _NC_CACHE["kscale"] = s

    ident = np.eye(128, dtype=f16)
    in_maps = []
    for core in range(NCORES):
        n, hb = divmod(core, 2)
        h0 = hb * HSH
        # xt[p=(c,wb), i, j] = xp[n, c, h0+j, wb*16+i]
        xv = xp[n][:, h0:h0 + XJ, :]              # (C, 132, 260)
        blocks = np.lib.stride_tricks.sliding_window_view(
            xv, XI, axis=2)[:, :, ::WBW, :]       # (C, 132, 16, 20)
        xt = blocks.transpose(0, 2, 3, 1).reshape(128, XI * XJ)
        # kt[t, p=(c,wb), (i,j)] = kr[n, c, t, h0+j, wb*16+i]
        kc = kr[n][:, :, h0:h0 + HSH, :]          # (C, 25, 128, 256)
        kt = kc.transpose(1, 0, 3, 2)             # (25, C, 256, 128)
        kt = kt.reshape(NT, C, WB, WBW, HSH).reshape(NT, 128, FREE)
        kf = kt[list(F_TAPS)].transpose(1, 0, 2)          # [128, NF, FREE]
        kq = np.clip(np.rint(kt[list(Q_TAPS)] / s), -127, 127) \
            .astype(np.int8).transpose(1, 0, 2)           # [128, NQ, FREE]
        in_maps.append({"xt": np.ascontiguousarray(xt), "ident": ident,
                        "kf": np.ascontiguousarray(kf.astype(f16)),
                        "kq": np.ascontiguousarray(kq)})
    return in_maps


def kernel(x: np.ndarray, kernel: np.ndarray) -> np.ndarray:
    in_maps = _prep_shards(np.asarray(x), np.asarray(kernel))
    nc = get_nc()
    trace = bool(int(os.environ.get("KC_TRACE", "0")))
    res = run_bass_kernel_spmd(nc, in_maps, core_ids=list(range(NCORES)),
                               trace=trace)
    _NC_CACHE["last_results"] = res
    out = np.empty((N, C, H, W), np.float32)
    for core in range(NCORES):
        n, hb = divmod(core, 2)
        h0 = hb * HSH
        o = res.results[core]["out"].reshape(C, WB, WBW, HSH)
        out[n, :, h0:h0 + HSH, :] = \
            o.transpose(0, 3, 1, 2).reshape(C, HSH, W).astype(np.float32)
    return out


# revision 3
# speedup vs baseline: 1.0937x; 1.0746x over previous
"""Per-pixel dynamic 5x5 conv (KernelConv2d) + leaky-relu, data-parallel on 8 TRN2 cores.

v4 architecture:
  Sharding: core i <- (n = i//2, h-half = i%2); each core computes out[n,:,h0:h0+128,:].
  On-chip layout: partitions = (c, w-block16); free = (w-in-block, h). The x tile
  stores a 4-col/4-row halo per partition ([128, 20, 132]) so every tap (dy,dx) is a
  free-dim shifted view -- x is DMA'd once (0.68 MB), no window copies.
  DVE does ONLY the 25 products (f16 tensor_tensor); the 24 adds run on the idle PE
  (identity-stationary matmuls accumulating in PSUM fp32; PE is pre-warmed with
  dummy matmuls to ramp its clock); the lrelu tail is ACT psum->f16 copy + one DVE
  scalar_tensor_tensor max(0.2g, g), split in halves to overlap the output DMA.
  The kernel stream is rate-matched to the DMA queue bus (22.5 B/ns x 16 queues):
  20 taps ship as f16 (DMA'd in pairs to keep the DMA/semaphore count and program
  preamble small) and 5 interspersed taps ship as int8 (half the bytes), upcast to
  f16 by the idle ACT engine with the dequant scale folded into the copy, so byte
  arrival keeps pace with DVE consumption. int8 rel-err ~5e-3 (gate 2e-2).
"""

import os
from contextlib import ExitStack

import numpy as np

import concourse.bass as bass
import concourse.mybir as mybir
from concourse.bass_utils import run_bass_kernel_spmd

N, C, H, W = 4, 8, 256, 256
K = 5
PAD = 2
NCORES = 8
HSH = H // 2            # 128 output rows per core
WB, WBW = 16, 16        # 16 w-blocks of 16 columns
XI, XJ = WBW + 2 * PAD, HSH + 2 * PAD   # 20 x 132 halo'd tile per partition
NT = K * K              # 25 taps
FREE = WBW * HSH        # 2048 product elems per partition
HALF = FREE // 2
BANK = 512              # PSUM fp32 bank width
RING = 10               # product ring buffers
WARM = 12               # PE warm-up matmuls

F16 = mybir.dt.float16
F32 = mybir.dt.float32
I8 = mybir.dt.int8

Q_TAPS = (4, 9, 14, 19, 22)         # int8 taps, upcast by ACT
F_TAPS = tuple(t for t in range(NT) if t not in Q_TAPS)
NF, NQ = len(F_TAPS), len(Q_TAPS)
F_GROUPS = ([(F_TAPS[0],), (F_TAPS[1],)]
            + [F_TAPS[i:i + 2] for i in range(2, NF, 2)])  # 1,1,2,2,... DMAs
NEG = 0.2

_NC_CACHE = {}


def _build_nc():
    nc = bass.Bass("TRN2", target_bir_lowering=False, debug=False,
                   num_devices=NCORES)
    xt_d = nc.dram_tensor("xt", [128, XI * XJ], F16, kind="ExternalInput").ap()
    id_d = nc.dram_tensor("ident", [128, 128], F16, kind="ExternalInput").ap()
    kf_d = nc.dram_tensor("kf", [128, NF, FREE], F16, kind="ExternalInput").ap()
    kq_d = nc.dram_tensor("kq", [128, NQ, FREE], I8, kind="ExternalInput").ap()
    out_d = nc.dram_tensor("out", [128, FREE], F16, kind="ExternalOutput").ap()

    taps = [(dy, dx) for dy in range(K) for dx in range(K)]
    fidx = {t: i for i, t in enumerate(F_TAPS)}
    qidx = {t: i for i, t in enumerate(Q_TAPS)}

    with ExitStack() as ctx:
        xt = ctx.enter_context(nc.sbuf_tensor("xts", [128, XI, XJ], F16))
        ident = ctx.enter_context(nc.sbuf_tensor("ids", [128, 128], F16))
        kbf = ctx.enter_context(nc.sbuf_tensor("kbf", [128, NF, FREE], F16))
        kb8 = ctx.enter_context(nc.sbuf_tensor("kb8", [128, NQ, FREE], I8))
        kbq = ctx.enter_context(nc.sbuf_tensor("kbq", [128, NQ, FREE], F16))
        pr = [ctx.enter_context(nc.sbuf_tensor(f"pr{r}", [128, FREE], F16))
              for r in range(RING)]
        gt = ctx.enter_context(nc.sbuf_tensor("gt", [128, FREE], F16))
        ot = ctx.enter_context(nc.sbuf_tensor("ot", [128, FREE], F16))
        ps = ctx.enter_context(nc.psum_tensor("ps", [128, FREE], F32))
        s_x = ctx.enter_context(nc.semaphore("s_x"))
        s_id = ctx.enter_context(nc.semaphore("s_id"))
        s_g = [ctx.enter_context(nc.semaphore(f"sg{i}"))
               for i in range(len(F_GROUPS))]
        s_q8 = [ctx.enter_context(nc.semaphore(f"sq{i}")) for i in range(NQ)]
        s_ku = ctx.enter_context(nc.semaphore("s_ku"))
        s_pv = ctx.enter_context(nc.semaphore("s_pv"))
        s_free = ctx.enter_context(nc.semaphore("s_free"))
        s_mm0 = ctx.enter_context(nc.semaphore("s_mm0"))
        s_mm1 = ctx.enter_context(nc.semaphore("s_mm1"))
        s_cp = ctx.enter_context(nc.semaphore("s_cp"))
        s_lr = ctx.enter_context(nc.semaphore("s_lr"))
        s_o = ctx.enter_context(nc.semaphore("s_o"))
        block = ctx.enter_context(nc.Block())

        @block.sync
        def _(sync):
            sync.dma_start(xt[:], xt_d.rearrange("p (i j) -> p i j", i=XI)) \
                .then_inc(s_x, 16)
            sync.dma_start(ident[:], id_d).then_inc(s_id, 16)
            # interleave: one int8 DMA after every second f16 pair
            qi = 0
            for g, pair in enumerate(F_GROUPS):
                f0 = fidx[pair[0]]
                sync.dma_start(kbf[:, f0:f0 + len(pair)],
                               kf_d[:, f0:f0 + len(pair)]) \
                    .then_inc(s_g[g], 16)
                if g >= 1 and qi < NQ:
                    sync.dma_start(kb8[:, qi:qi + 1], kq_d[:, qi:qi + 1]) \
                        .then_inc(s_q8[qi], 16)
                    qi += 1
            sync.wait_ge(s_lr, 1)
            sync.dma_start(out_d[:, :HALF], ot[:, :HALF]).then_inc(s_o, 16)
            sync.wait_ge(s_lr, 2)
            sync.dma_start(out_d[:, HALF:], ot[:, HALF:]).then_inc(s_o, 16)
            sync.wait_ge(s_o, 32)

        @block.scalar
        def _(scalar):
            # preload the Copy act table off the critical path
            scalar.wait_ge(s_id, 16)
            scalar.activation(gt[:, :1], ident[:, :1],
                              mybir.ActivationFunctionType.Copy, scale=1.0)
            sq = float(_NC_CACHE.get("kscale", 1.0))
            for qi in range(NQ):
                scalar.wait_ge(s_q8[qi], 16)
                scalar.activation(kbq[:, qi], kb8[:, qi],
                                  mybir.ActivationFunctionType.Copy,
                                  scale=sq).then_inc(s_ku, 1)
            # tail: psum -> f16, in halves
            scalar.wait_ge(s_mm0, 1)
            scalar.activation(gt[:, :HALF], ps[:, :HALF],
                              mybir.ActivationFunctionType.Copy,
                              scale=1.0).then_inc(s_cp, 1)
            scalar.wait_ge(s_mm1, 1)
            scalar.activation(gt[:, HALF:], ps[:, HALF:],
                              mybir.ActivationFunctionType.Copy,
                              scale=1.0).then_inc(s_cp, 1)

        @block.vector
        def _(vector):
            vector.wait_ge(s_x, 16)
            for t, (dy, dx) in enumerate(taps):
                if t in qidx:
                    vector.wait_ge(s_ku, qidx[t] + 1)
                    kv = kbq[:, qidx[t]]
                else:
                    fi = fidx[t]
                    g = fi if fi < 2 else 2 + (fi - 2) // 2
                    vector.wait_ge(s_g[g], 16)
                    kv = kbf[:, fi]
                if t >= RING:
                    vector.wait_ge(s_free, t - RING + 1)
                xs = xt[:, dx:dx + WBW, dy:dy + HSH]
                prv = pr[t % RING].ap().rearrange("p (i j) -> p i j", i=WBW)
                vector.tensor_tensor(prv, xs,
                                     kv.rearrange("p (i j) -> p i j", i=WBW),
                                     op=mybir.AluOpType.mult).then_inc(s_pv, 1)
            # lrelu tail halves: ot = max(NEG*gt, gt)
            for h in range(2):
                vector.wait_ge(s_cp, h + 1)
                sl = slice(h * HALF, (h + 1) * HALF)
                vector.scalar_tensor_tensor(ot[:, sl], gt[:, sl], NEG, gt[:, sl],
                                            op0=mybir.AluOpType.mult,
                                            op1=mybir.AluOpType.max) \
                    .then_inc(s_lr, 1)

        @block.tensor
        def _(tensor):
            tensor.wait_ge(s_id, 16)
            tensor.wait_ge(s_x, 16)
            for w in range(WARM):
                tensor.matmul(ps[:, 0:BANK], ident[:],
                              xt.ap().rearrange("p i j -> p (i j)")[:, 0:BANK],
                              start=True, stop=True)
            for t in range(NT):
                tensor.wait_ge(s_pv, t + 1)
                prf = pr[t % RING]
                for j in range(4):
                    mm = tensor.matmul(ps[:, j * BANK:(j + 1) * BANK], ident[:],
                                       prf[:, j * BANK:(j + 1) * BANK],
                                       start=(t == 0), stop=(t == NT - 1))
                    if t == NT - 1 and j == 1:
                        mm.then_inc(s_mm0, 1)
                if t == NT - 1:
                    mm.then_inc(s_mm1, 1)
                else:
                    mm.then_inc(s_free, 1)

    return nc


def get_nc():
    if "nc" not in _NC_CACHE:
        _NC_CACHE["nc"] = _build_nc()
    return _NC_CACHE["nc"]


def _prep_shards(x: np.ndarray, kernel: np.ndarray):
    f16 = np.float16
    xp = np.pad(x, ((0, 0), (0, 0), (PAD, PAD), (PAD, PAD)),
                mode='edge').astype(f16)          # (N, C, 260, 260)
    kr = kernel.reshape(N, C, NT, H, W)

    s = float(np.abs(kernel).max()) / 127.0
    _NC_CACHE["kscale"] = s

    ident = np.eye(128, dtype=f16)
    in_maps = []
    for core in range(NCORES):
        n, hb = divmod(core, 2)
        h0 = hb * HSH
        # xt[p=(c,wb), i, j] = xp[n, c, h0+j, wb*16+i]
        xv = xp[n][:, h0:h0 + XJ, :]              # (C, 132, 260)
        blocks = np.lib.stride_tricks.sliding_window_view(
            xv, XI, axis=2)[:, :, ::WBW, :]       # (C, 132, 16, 20)
        xt = blocks.transpose(0, 2, 3, 1).reshape(128, XI * XJ)
        # kt[t, p=(c,wb), (i,j)] = kr[n, c, t, h0+j, wb*16+i]
        kc = kr[n][:, :, h0:h0 + HSH, :]          # (C, 25, 128, 256)
        kt = kc.transpose(1, 0, 3, 2)             # (25, C, 256, 128)
        kt = kt.reshape(NT, C, WB, WBW, HSH).reshape(NT, 128, FREE)
        kf = kt[list(F_TAPS)].transpose(1, 0, 2)          # [128, NF, FREE]
        kq = np.clip(np.rint(kt[list(Q_TAPS)] / s), -127, 127) \
            .astype(np.int8).transpose(1, 0, 2)           # [128, NQ, FREE]
        in_maps.append({"xt": np.ascontiguousarray(xt), "ident": ident,
                        "kf": np.ascontiguousarray(kf.astype(f16)),
                        "kq": np.ascontiguousarray(kq)})
    return in_maps


def kernel(x: np.ndarray, kernel: np.ndarray) -> np.ndarray:
    in_maps = _prep_shards(np.asarray(x), np.asarray(kernel))
    nc = get_nc()
    trace = bool(int(os.environ.get("KC_TRACE", "0")))
    res = run_bass_kernel_spmd(nc, in_maps, core_ids=list(range(NCORES)),
                               trace=trace)
    _NC_CACHE["last_results"] = res
    out = np.empty((N, C, H, W), np.float32)
    for core in range(NCORES):
        n, hb = divmod(core, 2)
        h0 = hb * HSH
        o = res.results[core]["out"].reshape(C, WB, WBW, HSH)
        out[n, :, h0:h0 + HSH, :] = \
            o.transpose(0, 3, 1, 2).reshape(C, HSH, W).astype(np.float32)
    return out
